# revision 28
# baseline (speedup 1.0000x reference)
"""Trainium2 Bass kernel for nn_CATransformer1 (XCiT-style channel-attention block).

Sharding: data-parallel over batch. 16 images / 8 cores = 2 images per core.
Weights replicated; no collectives.

V3 design (matmul cost on TRN2 = out-free-size x pe_cycle x cycles_per_row,
independent of K/M; fp8e4+DoubleRow = 0.5 cycles/row):
  - Gram trick: per head, one matmul lhsT=rhs=[q|k] accumulates S=q^T k AND
    both norm^2 diagonals; kills qksq (DVE), k-norm row (F=384/chunk) and
    q-norm (F=1 x8/chunk) entirely.  Diagonals extracted once per image via
    tensor_tensor_reduce against identity.
  - LN1 mean fold applied at eviction on DVE (scalar_tensor_tensor with a
    host-broadcast usb=ones(x)u) instead of K=1 rank-1 matmuls on the PE.
  - All ones(x)row PE broadcasts replaced with gpsimd partition_broadcast.
  - FFN1 zero-padded to 4 K-subtiles -> pure fp8 DoubleRow.
  - Optional fp8 qkT and fp8 G-apply (DoubleRow, zero-padded 4th subtile).
  - reciprocal_approx_fast everywhere (LN rstd, norms, softmax sum).
  - Softmax max-subtraction dropped (|logits| <= ~10, exp safe in f32).
  - y computed in-place into the SBUF-resident x (no HBM re-read in B1);
    FFN (B2) chunks interleaved into the B1 pipeline to keep the PE fed.
"""

import numpy as np

B, C, NH, CH, N, HID = 16, 384, 8, 48, 4096, 1536
NCORES = 8
BPC = B // NCORES  # images per core
P = 128
KS = C // P   # 3 k-subtiles for C
KH = HID // P  # 12 k-subtiles for HID
NT = N // P   # 32 pixel chunks (phase A)
FG = 512      # phase B pixel chunk
NFG = N // FG
LOGIT_MAX = float(np.log(1.0 / 0.01))
EPS_LN = 1e-5
EPS_NORM = 1e-12

FP8_QK = False   # qkT matmul in fp8 DoubleRow (x_f8 = 8x, wqk_f8 = 64w)
FP8_G = False    # G-apply + s3 in fp8 DoubleRow

XSC = 8.0        # x fp8 scale
WSC = 64.0       # weight fp8 scale
GSC = 256.0      # G fp8 scale

_CACHE = {}


def _patch_tile_drain():
    """Walrus in this env rejects >1 sync-wait on the kernel-tail Drain
    (CTRL_NO_STRUCT setupSyncWait).  Split the waits across a chain of
    drain instructions, one wait each.  Idempotent, in-process only."""
    import concourse.tile as tile
    from concourse import mybir
    from concourse.vector_clock import ScopedClock

    if getattr(tile.TileContext._drain_and_barrier, "_split_patch", False):
        return

    def _split_drain(self, tick_clock, wait_clock):
        drain_inst = self.nc.sync.drain()
        wait_clock.add_sem_waits(
            drain_inst.ins, ScopedClock({None: tick_clock.global_clock}))
        si = drain_inst.ins.sync_info
        if si is not None and si.on_wait and len(si.on_wait) > 1:
            waits = list(si.on_wait)
            si.on_wait = waits[:1]
            for w in waits[1:]:
                d2 = self.nc.sync.drain()
                d2.ins.sync_info = mybir.SyncInfo(on_wait=[w], on_update=[])
        self.nc.all_engine_barrier()
        popped = self.nc._tile_sem_poison_stack.pop()
        assert popped is self._sem_poison
        self.nc.clear_and_free_semaphores(list(self.sems.allocated().values()))
        self.nc.all_engine_barrier()

    _split_drain._split_patch = True
    tile.TileContext._drain_and_barrier = _split_drain


def _split_waits(nc, max_waits=1):
    """This walrus build rejects instructions carrying more than one sync
    wait ('Too many sync wait commands' / 'ISA wrong length').  Move extra
    waits onto same-engine NoOps inserted immediately before."""
    from concourse import mybir

    n = 0
    for fn in nc.m.functions:
        for blk in fn.blocks:
            out = []
            for inst in blk.instructions:
                si = inst.sync_info
                # custom-DVE InstISA can't carry any sync commands at all
                mw = 0 if isinstance(inst, mybir.InstISA) else max_waits
                if si is not None and si.on_wait and len(si.on_wait) > mw:
                    waits = list(si.on_wait)
                    keep = waits[-mw:] if mw else []
                    for w in waits[:len(waits) - mw]:
                        n += 1
                        nop = mybir.InstNoOp(
                            name=f"I-wsplit-{n}", ins=[], outs=[])
                        nop.engine = inst.engine
                        nop.sync_info = mybir.SyncInfo(
                            on_wait=[w], on_update=[])
                        out.append(nop)
                    si.on_wait = keep
                out.append(inst)
                if (isinstance(inst, mybir.InstISA) and si is not None
                        and si.on_update):
                    n += 1
                    nop = mybir.InstNoOp(name=f"I-usplit-{n}", ins=[], outs=[])
                    nop.engine = inst.engine
                    nop.sync_info = mybir.SyncInfo(
                        on_wait=[], on_update=list(si.on_update))
                    out.append(nop)
                    si.on_update = []
            blk.instructions = out
    return nc


def _build_nc():
    import concourse.bass as bass
    import concourse.tile as tile
    from concourse import mybir

    dt = mybir.dt
    AF = mybir.ActivationFunctionType
    ALU = mybir.AluOpType
    AX = mybir.AxisListType
    from concourse.masks import make_identity

    f32 = dt.float32
    bf16 = dt.bfloat16
    f8 = dt.float8e4

    _patch_tile_drain()
    nc = bass.Bass()

    xs = nc.declare_dram_parameter("xs", [BPC, C, N], bf16, isOutput=False)
    if FP8_QK:
        xs8 = nc.declare_dram_parameter("xs8", [BPC, C, N], f8, isOutput=False)
        wqk_t = nc.declare_dram_parameter("wqk_t", [4 * P, 2 * C], f8,
                                          isOutput=False)
    else:
        wqk_t = nc.declare_dram_parameter("wqk_t", [C, 2 * C], bf16,
                                          isOutput=False)
    usb_d = nc.declare_dram_parameter("usb", [P, 2 * C], bf16, isOutput=False)
    wv = nc.declare_dram_parameter("wv", [CH, NH, C], bf16, isOutput=False)
    if FP8_G:
        wpj48 = nc.declare_dram_parameter("wpj48", [CH, NH, C], f8,
                                          isOutput=False)
    else:
        wpj48 = nc.declare_dram_parameter("wpj48", [CH, NH, C], bf16,
                                          isOutput=False)
    w1_t = nc.declare_dram_parameter("w1_t", [4 * P, HID], f8, isOutput=False)
    w2_t = nc.declare_dram_parameter("w2_t", [HID, C], f8, isOutput=False)
    scale_row = nc.declare_dram_parameter("scale_row", [1, NH], f32,
                                          isOutput=False)
    out_d = nc.declare_dram_parameter("out", [BPC, C, N], bf16, isOutput=True)
    # DRAM scratch for partition-broadcast round trips: a row written here
    # comes back replicated across 128 partitions via a 0-stride DMA read
    rows_d = nc.dram_tensor("rows_sc", [BPC, 2, N], bf16)   # (rrow, mrnrow)
    rows2_d = nc.dram_tensor("rows2_sc", [2, N], bf16)      # LN2 (m2, r2)

    qk_dt = f8 if FP8_QK else bf16
    g_dt = f8 if FP8_G else bf16
    KSQ = 4 if FP8_QK else KS   # qkT lhs subtiles incl zero pad
    KSG = 4 if FP8_G else KS

    with tile.TileContext(nc) as tc:
        with (
            tc.tile_pool(name="consts", bufs=1) as consts,
            tc.tile_pool(name="ximg", bufs=2) as xpool,
            tc.tile_pool(name="qkp", bufs=2) as qkpool,
            tc.tile_pool(name="attn", bufs=2) as apool,
            tc.tile_pool(name="scr", bufs=3) as scr,
            tc.tile_pool(name="bwork", bufs=2) as bw,
            tc.tile_pool(name="pb", bufs=6, space="PSUM") as ps,
            tc.tile_pool(name="acc", bufs=1, space="PSUM") as psacc,
        ):
            def bcast_read(dst, dram_row, parts):
                src = bass.AP(
                    tensor=dram_row.tensor, offset=dram_row.offset,
                    ap=[[0, parts]] + [list(d) for d in dram_row.ap[-1:]])
                nc.gpsimd.dma_start(dst, src)

            # ---------------- constants ----------------
            ones_col = consts.tile([P, 1], bf16, tag="onescol")
            nc.vector.memset(ones_col[:], 1.0)
            ones_row = consts.tile([1, P], bf16, tag="onesrow")
            nc.vector.memset(ones_row[:], 1.0)
            identb = consts.tile([P, P], bf16, tag="identb")
            make_identity(nc, identb[:])
            schb = consts.tile([CH, NH], f32, tag="schb")
            bcast_read(schb[:], scale_row[0, :], parts=CH)

            xs_r = xs.rearrange("b (s p) n -> b p s n", p=P)
            out_r = out_d.rearrange("b (s p) n -> b p s n", p=P)

            # x images: first 512-px chunk of img0 goes first (gates phase A)
            x_tiles, rowpairs = [], []
            for img in range(BPC):
                x_sb = xpool.tile([P, KS, N], bf16, tag="x")
                x_tiles.append(x_sb)
                # rows[0] = rstd, rows[1] = -mean*rstd (per pixel)
                rows = xpool.tile([1, 2, N], bf16, tag="rows")
                rowpairs.append(rows)
            nc.sync.dma_start(x_tiles[0][:, :, 0:512], xs_r[0][:, :, 0:512])

            # qkv weights split across queues for fast availability
            wqk_sb = consts.tile([P, KSQ, 2 * C], qk_dt, tag="wqk")
            wqk_r = wqk_t.rearrange("(s p) f -> p s f", p=P)
            nc.scalar.dma_start(wqk_sb[:, 0, :], wqk_r[:, 0, :])
            nc.gpsimd.dma_start(wqk_sb[:, 1, :], wqk_r[:, 1, :])
            nc.gpsimd.dma_start(wqk_sb[:, 2, :], wqk_r[:, 2, :])
            if FP8_QK:
                nc.scalar.dma_start(wqk_sb[:, 3, :], wqk_r[:, 3, :])
            usb = consts.tile([P, 2 * C], bf16, tag="usb")
            nc.scalar.dma_start(usb[:], usb_d[:])

            # fp8 copies of x (4th subtile zeroed once; pairs with zero rows
            # in wqk so garbage would be harmless, but keep it clean)
            if FP8_QK:
                xs8_r = xs8.rearrange("b (s p) n -> b p s n", p=P)
                x8_tiles = []
                for img in range(BPC):
                    x8 = xpool.tile([P, 4, N], f8, tag="x8")
                    nc.gpsimd.memset(x8[:, 3, :], 0.0)
                    for i in range(4):
                        sl = slice(i * 1024, (i + 1) * 1024)
                        nc.scalar.dma_start(x8[:, 0:3, sl], xs8_r[img][:, :, sl])
                    x8_tiles.append(x8)

            # rest of x img0 + img1
            for img in range(BPC):
                for i in range(8):
                    if img == 0 and i == 0:
                        continue
                    sl = slice(i * 512, (i + 1) * 512)
                    nc.sync.dma_start(x_tiles[img][:, :, sl],
                                      xs_r[img][:, :, sl])

            wv_sb = consts.tile([CH, NH, C], bf16, tag="wv")
            nc.gpsimd.dma_start(wv_sb[:], wv[:])
            wpj_sb = consts.tile([CH, NH, C], g_dt, tag="wpj")
            nc.gpsimd.dma_start(wpj_sb[:], wpj48[:])
            w1_sb = consts.tile([P, 4, HID], f8, tag="w1")
            nc.gpsimd.dma_start(w1_sb[:], w1_t.rearrange("(s p) f -> p s f", p=P))
            w2_sb = consts.tile([P, KH, C], f8, tag="w2")
            nc.gpsimd.dma_start(w2_sb[:], w2_t.rearrange("(s p) f -> p s f", p=P))

            # yn ring: LN2 output only lives from ln2_apply(f) to b2(f), so a
            # 3-deep ring of single-chunk tiles suffices.  4th subtile is the
            # DoubleRow zero pad, written once (never touched again).
            yns = []
            for i in range(3):
                ynt = bw.tile([P, 4, FG], f8, tag="yn", bufs=3,
                              name=f"ynring{i}")
                nc.vector.memset(ynt[:, 3, :], 0.0)
                yns.append(ynt)

            # ---------------- phase A ----------------
            def alloc_acc():
                # per-head Gram accumulator [q|k]^T[q|k], 128-col stride so
                # every head's 96x96 block stays inside one PSUM bank
                return psacc.tile([2 * CH, NH, P], f32, tag="accg",
                                  name="accg")

            def phase_a(img, accg, interleave=()):
                x_sb = x_tiles[img]
                rows = rowpairs[img]

                def stats_mm2(t):
                    """Batched LN1 stats for chunks t and t+1."""
                    sl = slice(t * P, (t + 2) * P)
                    xsq = scr.tile([P, KS, 2 * P], bf16, tag="xsq", bufs=2)
                    nc.gpsimd.tensor_mul(xsq[:], x_sb[:, :, sl], x_sb[:, :, sl])
                    pstat = ps.tile([P, 2, 2], f32, tag="pb")
                    for cp in range(2):
                        csl = slice((t + cp) * P, (t + cp + 1) * P)
                        for s in range(KS):
                            nc.tensor.matmul(
                                pstat[:, cp, 0:1], x_sb[:, s, csl], ones_col[:],
                                start=(s == 0), stop=(s == KS - 1))
                        for s in range(KS):
                            nc.tensor.matmul(
                                pstat[:, cp, 1:2],
                                xsq[:, s, cp * P:(cp + 1) * P], ones_col[:],
                                start=(s == 0), stop=(s == KS - 1))
                    # stat2 cols: 0 = rstd, 32 = -m*rstd (transposed to rows)
                    stat2 = scr.tile([P, 2, 33], bf16, tag="stat2", bufs=2)
                    mcol = scr.tile([P, 2], f32, tag="mcol", bufs=2)
                    vcol = scr.tile([P, 2], f32, tag="vcol")
                    msq = scr.tile([P, 2], f32, tag="msq")
                    rcol = scr.tile([P, 2], f32, tag="rcol", bufs=2)
                    mrn = scr.tile([P, 2], f32, tag="mrn", bufs=2)
                    nc.scalar.activation(mcol[:], pstat[:, :, 0], AF.Copy,
                                         scale=-1.0 / C)
                    nc.vector.tensor_scalar(
                        vcol[:], pstat[:, :, 1], 1.0 / C, EPS_LN,
                        op0=ALU.mult, op1=ALU.add)
                    nc.scalar.activation(msq[:], mcol[:], AF.Square)
                    nc.vector.tensor_sub(vcol[:], vcol[:], msq[:])
                    nc.scalar.activation(vcol[:], vcol[:], AF.Sqrt)
                    nc.vector.reciprocal(rcol[:], vcol[:])
                    nc.vector.tensor_copy(stat2[:, :, 0], rcol[:])
                    nc.vector.tensor_mul(mrn[:], mcol[:], rcol[:])
                    nc.vector.tensor_copy(stat2[:, :, 32], mrn[:])
                    if FP8_QK:
                        # usb is host-scaled by XSC*WSC to match pa's scale;
                        # the final r-scale compensates
                        rcolq = scr.tile([P, 2], f32, tag="rcolq", bufs=2)
                        nc.vector.tensor_scalar_mul(
                            rcolq[:], rcol[:], 1.0 / (XSC * WSC))
                        return stat2, rcolq, mcol
                    return stat2, rcol, mcol

                def stats_tr(t, stat2, cp):
                    sl = slice((t + cp) * P, (t + cp + 1) * P)
                    ptr = ps.tile([33, P], bf16, tag="pb")
                    nc.tensor.transpose(ptr[:], stat2[:, cp, :], identb[:])
                    nc.scalar.copy(rows[0:1, 0, sl], ptr[0:1, :])
                    nc.scalar.copy(rows[0:1, 1, sl], ptr[32:33, :])

                pend = None
                cur = stats_mm2(0)
                stats_tr(0, cur[0], 0)
                stats_tr(0, cur[0], 1)
                nxt = None
                for t in range(NT):
                    if 1 <= t <= len(interleave):
                        interleave[t - 1]()
                    sl = slice(t * P, (t + 1) * P)
                    cp = t % 2
                    rcol_t = cur[1][:, cp:cp + 1]
                    mcol_t = cur[2][:, cp:cp + 1]
                    if t % 2 == 0 and t + 2 < NT:
                        nxt = stats_mm2(t + 2)
                    # qkT x-part into PSUM (two banks)
                    pa1 = ps.tile([P, 512], f32, tag="pb")
                    pa2 = ps.tile([P, 256], f32, tag="pb")
                    if FP8_QK:
                        x8 = x8_tiles[img]
                        for sp in range(2):
                            ss = slice(2 * sp, 2 * sp + 2)
                            nc.tensor.matmul(
                                pa1[:], x8[:, ss, sl], wqk_sb[:, ss, 0:512],
                                start=(sp == 0), stop=(sp == 1),
                                perf_mode=mybir.MatmulPerfMode.DoubleRow)
                        for sp in range(2):
                            ss = slice(2 * sp, 2 * sp + 2)
                            nc.tensor.matmul(
                                pa2[:], x8[:, ss, sl], wqk_sb[:, ss, 512:768],
                                start=(sp == 0), stop=(sp == 1),
                                perf_mode=mybir.MatmulPerfMode.DoubleRow)
                    else:
                        for s in range(KS):
                            nc.tensor.matmul(
                                pa1[:], x_sb[:, s, sl], wqk_sb[:, s, 0:512],
                                start=(s == 0), stop=(s == KS - 1))
                        for s in range(KS):
                            nc.tensor.matmul(
                                pa2[:], x_sb[:, s, sl], wqk_sb[:, s, 512:768],
                                start=(s == 0), stop=(s == KS - 1))
                    if t % 2 == 0 and t + 2 < NT:
                        stats_tr(t + 2, nxt[0], 0)
                        stats_tr(t + 2, nxt[0], 1)
                    # deferred S/norm accumulation from previous chunk
                    if pend is not None:
                        _emit_s(accg, pend[0], pend[1])
                    # eviction: qk = (pa + (-m)*u) * r
                    # z = usb*(-m) + pa on DVE; qk = z*r on ACT
                    z = qkpool.tile([P, 2 * C], bf16, tag="z")
                    qk = qkpool.tile([P, 2 * C], bf16, tag="qk")
                    nc.vector.scalar_tensor_tensor(
                        z[:, 0:512], usb[:, 0:512], mcol_t, pa1[:],
                        op0=ALU.mult, op1=ALU.add)
                    nc.vector.scalar_tensor_tensor(
                        z[:, 512:768], usb[:, 512:768], mcol_t, pa2[:],
                        op0=ALU.mult, op1=ALU.add)
                    nc.scalar.activation(qk[:], z[:], AF.Copy, scale=rcol_t)
                    pend = (qk, t)
                    if t % 2 == 1:
                        cur = nxt
                _emit_s(accg, pend[0], pend[1])
                # stage the per-pixel LN1 rows in DRAM for phase-B broadcasts
                nc.scalar.dma_start(rows_d[img], rows[0:1, :, :])

            def _emit_s(accg, qk, t):
                st, sp = (t == 0), (t == NT - 1)
                for h in range(NH):
                    o = h * 2 * CH
                    nc.tensor.matmul(
                        accg[:, h, 0:2 * CH],
                        qk[:, o:o + 2 * CH], qk[:, o:o + 2 * CH],
                        start=st, stop=sp)

            # ---------------- attention stages ----------------
            def attn_stages(img, accg):
                st = {}

                def s0():  # norms + scaled S + softmax -> sSb (bf16)
                    # free the PSUM accumulator early: S copy + diag extract
                    sraw = apool.tile([CH, NH, CH], f32, tag="sraw", bufs=1)
                    s_v = accg[0:CH, :, CH:2 * CH]
                    nc.vector.tensor_copy(sraw[:], s_v)
                    nsq = apool.tile([2 * CH, NH], f32, tag="nsq", bufs=1)
                    dscr = apool.tile([2 * CH, 2 * CH], bf16, tag="dscr",
                                      bufs=1)
                    for h in range(NH):
                        nc.vector.tensor_mul(
                            dscr[:], accg[:, h, 0:2 * CH],
                            identb[0:2 * CH, 0:2 * CH])
                        nc.vector.reduce_sum(nsq[:, h:h + 1], dscr[:],
                                             axis=AX.X)
                    rinv = apool.tile([2 * CH, NH], f32, tag="rinv", bufs=1)
                    nc.scalar.activation(rinv[:], nsq[:], AF.Sqrt)
                    nc.vector.tensor_scalar_max(rinv[:], rinv[:], EPS_NORM)
                    nc.vector.reciprocal(rinv[:], rinv[:])
                    rq = apool.tile([CH, NH], f32, tag="rq", bufs=1)
                    nc.vector.tensor_mul(rq[:], rinv[0:CH, :], schb[:])
                    # engines can't address a partition window based at 48;
                    # DMA the k-half of rinv down to base 0
                    rkb32 = apool.tile([CH, NH], f32, tag="rkb32", bufs=1)
                    nc.gpsimd.dma_start(rkb32[:], rinv[CH:2 * CH, :])
                    rkb = apool.tile([CH, NH], bf16, tag="rkb", bufs=1)
                    nc.vector.tensor_copy(rkb[:], rkb32[:])
                    # flatten rk column [48, NH] to one row [1, 384] via 8
                    # single-column transposes, then partition-broadcast
                    prow = ps.tile([1, NH, CH], bf16, tag="pb")
                    for h in range(NH):
                        nc.tensor.transpose(
                            prow[:, h, :], rkb[:, h:h + 1], identb[0:CH, 0:CH])
                    rkrow = apool.tile([1, NH * CH], bf16, tag="rkrow", bufs=1)
                    nc.scalar.copy(rkrow[:], prow.rearrange("p h c -> p (h c)"))
                    rkps = ps.tile([CH, NH, CH], f32, tag="pb")
                    nc.tensor.matmul(
                        rkps.rearrange("p h c -> p (h c)"),
                        ones_row[0:1, 0:CH], rkrow[0:1, :],
                        start=True, stop=True)
                    sS = apool.tile([CH, NH, CH], f32, tag="sS", bufs=1)
                    nc.vector.tensor_mul(
                        sS[:], sraw[:], rq[:, :, None].to_broadcast(
                            (CH, NH, CH)))
                    nc.vector.tensor_mul(sS[:], sS[:], rkps[:])
                    # |logits| <= ~10: exp directly in f32, no max-sub
                    nc.scalar.activation(sS[:], sS[:], AF.Exp)
                    esum = apool.tile([CH, NH], f32, tag="esum", bufs=1)
                    nc.vector.reduce_sum(esum[:], sS[:], axis=AX.X)
                    nc.vector.reciprocal(esum[:], esum[:])
                    sSb = apool.tile([CH, NH, CH], bf16, tag="sSb", bufs=1)
                    nc.vector.tensor_mul(
                        sSb[:], sS[:],
                        esum[:, :, None].to_broadcast((CH, NH, CH)))
                    st["sSb"] = sSb

                def s1():  # transpose attn per head
                    pt8 = ps.tile([CH, NH, CH], bf16, tag="pb")
                    for h in range(NH):
                        nc.tensor.transpose(
                            pt8[:, h, :], st["sSb"][:, h, :], identb[0:CH, 0:CH])
                    atT = apool.tile([CH, NH, CH], bf16, tag="atT", bufs=1)
                    nc.vector.tensor_copy(atT[:], pt8[:])
                    st["atT"] = atT

                def s2():  # awv_h = attn_h @ Wv_h
                    awv = apool.tile([CH, NH, C], g_dt, tag="awv", bufs=1)
                    for h in range(NH):
                        paw = ps.tile([CH, C], f32, tag="pb")
                        nc.tensor.matmul(
                            paw[:], st["atT"][:, h, :], wv_sb[:, h, :],
                            start=True, stop=True)
                        sc = WSC if FP8_G else 1.0
                        if h % 2 == 0:
                            nc.vector.tensor_scalar_mul(awv[:, h, :], paw[:], sc)
                        else:
                            nc.scalar.activation(awv[:, h, :], paw[:], AF.Copy,
                                                 scale=sc)
                    st["awv"] = awv

                def s3():  # G^T  (fp8: awv*64 x wpj*64 -> evict * 256/4096)
                    gt_sb = apool.tile([P, KSG, C], g_dt, tag="gt")
                    if FP8_G:
                        nc.gpsimd.memset(gt_sb[:, 3, :], 0.0)
                    for j in range(KS):
                        pgt = ps.tile([P, C], f32, tag="pb")
                        if FP8_G:
                            for hp in range(NH // 2):
                                hs = slice(2 * hp, 2 * hp + 2)
                                nc.tensor.matmul(
                                    pgt[:], st["awv"][:, hs, j * P:(j + 1) * P],
                                    wpj_sb[:, hs, :],
                                    start=(hp == 0), stop=(hp == NH // 2 - 1),
                                    perf_mode=mybir.MatmulPerfMode.DoubleRow)
                        else:
                            for h in range(NH):
                                nc.tensor.matmul(
                                    pgt[:], st["awv"][:, h, j * P:(j + 1) * P],
                                    wpj_sb[:, h, :], start=(h == 0),
                                    stop=(h == NH - 1))
                        sc = GSC / (WSC * WSC) if FP8_G else 1.0
                        if j % 2 == 0:
                            nc.vector.tensor_scalar_mul(gt_sb[:, j, :], pgt[:],
                                                        sc)
                        else:
                            nc.scalar.activation(gt_sb[:, j, :], pgt[:],
                                                 AF.Copy, scale=sc)
                    st["gt"] = gt_sb

                def s4():  # uG column [128, KS] f32 (per out-channel block)
                    ugps = ps.tile([P, KS], f32, tag="pb")
                    onesb = ones_col
                    for j in range(KS):
                        for s in range(KS):
                            nc.tensor.matmul(
                                ugps[:, j:j + 1],
                                st["gt"][:, s, j * P:(j + 1) * P], onesb[:],
                                start=(s == 0), stop=(s == KS - 1))
                    ug = apool.tile([P, KS], f32, tag="ug")
                    # ug holds the true (unscaled) uG column
                    sc = 1.0 / GSC if FP8_G else 1.0
                    nc.vector.tensor_scalar_mul(ug[:], ugps[:], sc)
                    st["ug"] = ug

                return [s0, s1, s2, s3, s4], st

            # ---------------- phase B ----------------
            f8sc = 64.0  # host scales w1/w2 by 64
            g_inv = 1.0 / (GSC * XSC) if FP8_G else 1.0

            def bcast_sb(dst, row_ap, engine=None):
                """DRAM row -> SBUF replicated across partitions."""
                src = bass.AP(
                    tensor=row_ap.tensor, offset=row_ap.offset,
                    ap=[[0, dst.shape[0]]] + [list(d) for d in row_ap.ap])
                (engine or nc.sync).dma_start(dst, src)

            def b1_bcast(img, f):
                """Prefetch the (rstd, -m*rstd) broadcast for chunk f."""
                sl = slice(f * FG, (f + 1) * FG)
                brow = bw.tile([P, 2, FG], bf16, tag="brow", bufs=3)
                bcast_sb(brow, rows_d[img, :, sl], engine=nc.scalar)
                return brow

            def b1_chunk(img, st, f, brow):
                """Attn-branch apply + residual (in place into x) + ysq."""
                x_sb = x_tiles[img]
                gt, ug = st["gt"], st["ug"]
                sl = slice(f * FG, (f + 1) * FG)
                rb = brow[:, 0, :]    # rstd
                mrb = brow[:, 1, :]   # -m*rstd
                ab = bw.tile([P, KS, FG], bf16, tag="ab", bufs=2)
                for j in range(KS):
                    pg = ps.tile([P, FG], f32, tag="pb", name=f"pg{j}")
                    if FP8_G:
                        x8 = x8_tiles[img]
                        for sp in range(2):
                            ss = slice(2 * sp, 2 * sp + 2)
                            nc.tensor.matmul(
                                pg[:], gt[:, ss, j * P:(j + 1) * P],
                                x8[:, ss, sl], start=(sp == 0), stop=(sp == 1),
                                perf_mode=mybir.MatmulPerfMode.DoubleRow)
                    else:
                        for s in range(KS):
                            nc.tensor.matmul(
                                pg[:], gt[:, s, j * P:(j + 1) * P],
                                x_sb[:, s, sl], start=(s == 0),
                                stop=(s == KS - 1))
                    # pgr = (pg*g_inv)*rstd ; ab = mrb*uG[j] + pgr
                    pgr = bw.tile([P, FG], bf16, tag="pgr", bufs=3)
                    nc.vector.scalar_tensor_tensor(
                        pgr[:], pg[:], g_inv, rb,
                        op0=ALU.mult, op1=ALU.mult)
                    nc.vector.scalar_tensor_tensor(
                        ab[:, j, :], mrb, ug[:, j:j + 1], pgr[:],
                        op0=ALU.mult, op1=ALU.add)
                # y = x + ab, in place
                nc.gpsimd.tensor_add(x_sb[:, :, sl], x_sb[:, :, sl], ab[:])
                ysq = bw.tile([P, KS, FG], bf16, tag="ysq", bufs=2)
                nc.gpsimd.tensor_mul(ysq[:], x_sb[:, :, sl], x_sb[:, :, sl])
                return ysq

            def ln2_stats(img, f, ysq):
                x_sb = x_tiles[img]
                sl = slice(f * FG, (f + 1) * FG)
                p2a = ps.tile([1, FG], f32, tag="pb")
                p2b = ps.tile([1, FG], f32, tag="pb")
                for s in range(KS):
                    nc.tensor.matmul(
                        p2a[:], ones_col[:], x_sb[:, s, sl],
                        start=(s == 0), stop=(s == KS - 1))
                for s in range(KS):
                    nc.tensor.matmul(
                        p2b[:], ones_col[:], ysq[:, s, :],
                        start=(s == 0), stop=(s == KS - 1))
                m2r2 = scr.tile([1, 2, FG], bf16, tag="m2r2", bufs=2)
                nc.scalar.activation(m2r2[:, 0, :], p2a[:], AF.Copy,
                                     scale=-1.0 / C)
                vrow = scr.tile([1, FG], f32, tag="vrow", bufs=2)
                nc.vector.tensor_scalar(
                    vrow[:], p2b[:], 1.0 / C, EPS_LN, op0=ALU.mult, op1=ALU.add)
                msq = scr.tile([1, FG], f32, tag="msqr", bufs=2)
                nc.scalar.activation(msq[:], m2r2[:, 0, :], AF.Square)
                nc.vector.tensor_sub(vrow[:], vrow[:], msq[:])
                nc.scalar.activation(vrow[:], vrow[:], AF.Sqrt)
                r2f = scr.tile([1, FG], f32, tag="r2f", bufs=2)
                nc.vector.reciprocal(r2f[:], vrow[:])
                nc.scalar.copy(m2r2[:, 1, :], r2f[:])
                # round-trip through DRAM for the partition broadcast; write
                # and read are on the same queue so FIFO order is preserved
                nc.scalar.dma_start(rows2_d[:, sl], m2r2[0:1, :, :])
                brow2 = bw.tile([P, 2, FG], bf16, tag="brow2", bufs=2)
                bcast_sb(brow2, rows2_d[:, sl], engine=nc.scalar)
                return brow2

            def ln2_apply(img, f, brow2):
                x_sb = x_tiles[img]
                sl = slice(f * FG, (f + 1) * FG)
                yn = yns[f % 3]
                mbc = brow2[:, 0, :]
                rbc = brow2[:, 1, :]
                t3 = bw.tile([P, KS, FG], bf16, tag="t3", bufs=2)
                for s in range(KS):
                    nc.gpsimd.tensor_add(t3[:, s, :], x_sb[:, s, sl], mbc)
                for s in range(KS):
                    nc.gpsimd.tensor_mul(yn[:, s, :], t3[:, s, :], rbc)

            def b2_chunk(img, f):
                """One FFN chunk in fp8 DoubleRow + residual + store."""
                x_sb = x_tiles[img]
                sl = slice(f * FG, (f + 1) * FG)
                yn = yns[f % 3]
                h_sb = bw.tile([P, KH, FG], f8, tag="h", bufs=2)
                for mh in range(KH):
                    ph = ps.tile([P, FG], f32, tag="pb")
                    for sp in range(2):
                        ss = slice(2 * sp, 2 * sp + 2)
                        nc.tensor.matmul(
                            ph[:], w1_sb[:, ss, mh * P:(mh + 1) * P],
                            yn[:, ss, :], start=(sp == 0), stop=(sp == 1),
                            perf_mode=mybir.MatmulPerfMode.DoubleRow)
                    nc.scalar.activation(
                        h_sb[:, mh, :], ph[:], AF.Gelu, scale=1.0 / f8sc)
                o_sb = bw.tile([P, KS, FG], bf16, tag="o", bufs=2)
                for mo in range(KS):
                    po = ps.tile([P, FG], f32, tag="pb")
                    for sp in range(KH // 2):
                        nc.tensor.matmul(
                            po[:], w2_sb[:, 2 * sp:2 * sp + 2,
                                         mo * P:(mo + 1) * P],
                            h_sb[:, 2 * sp:2 * sp + 2, :],
                            start=(sp == 0), stop=(sp == KH // 2 - 1),
                            perf_mode=mybir.MatmulPerfMode.DoubleRow)
                    nc.vector.scalar_tensor_tensor(
                        o_sb[:, mo, :], po[:], 1.0 / f8sc, x_sb[:, mo, sl],
                        op0=ALU.mult, op1=ALU.add)
                nc.sync.dma_start(out_r[img][:, :, sl], o_sb[:])

            def phase_b(img, st, interleave=()):
                """B1 + LN2 + B2, chunk-pipelined: stats lag 1, apply/FFN
                lag 2 so the PE always has FFN work while DVE/gpsimd chains
                for the current chunk resolve."""
                ysqs, lrows = [], []
                brows = [b1_bcast(img, 0), b1_bcast(img, 1)]
                for f in range(NFG):
                    if f + 2 < NFG:
                        brows.append(b1_bcast(img, f + 2))
                    ysqs.append(b1_chunk(img, st, f, brows[f]))
                    if f < len(interleave):
                        interleave[f]()
                    if f >= 1:
                        lrows.append(ln2_stats(img, f - 1, ysqs[f - 1]))
                    if f >= 2:
                        ln2_apply(img, f - 2, lrows[f - 2])
                        b2_chunk(img, f - 2)
                lrows.append(ln2_stats(img, NFG - 1, ysqs[NFG - 1]))
                ln2_apply(img, NFG - 2, lrows[NFG - 2])
                b2_chunk(img, NFG - 2)
                ln2_apply(img, NFG - 1, lrows[NFG - 1])
                b2_chunk(img, NFG - 1)

            # ----------------- schedule -----------------
            acc0 = alloc_acc()
            phase_a(0, acc0)
            stages0, st0 = attn_stages(0, acc0)
            accB = alloc_acc()
            phase_a(1, accB, interleave=stages0)
            stages1, st1 = attn_stages(1, accB)
            phase_b(0, st0, interleave=stages1)
            phase_b(1, st1)

    # populate .instr bytes for extended-inst InstISA subclasses
    # (InstPartitionBroadcast / InstTensorTensorReduce / custom-DVE ops);
    # raw Bass skips Bacc.compile()'s codegen_inst_isa_subclasses pass
    from concourse.library_overlay import lower_extended_insts
    lower_extended_insts(nc)
    return _split_waits(nc)


def _prep_weights(inputs):
    import ml_dtypes
    bf = ml_dtypes.bfloat16
    f8 = ml_dtypes.float8_e4m3
    w_qkv = np.asarray(inputs["w_qkv"], np.float32)
    g1 = np.asarray(inputs["g1"], np.float32)
    g2 = np.asarray(inputs["g2"], np.float32)
    for name in ("beta1", "beta2", "b_qkv", "b_proj", "b_ffn1", "b_ffn2"):
        assert not np.any(np.asarray(inputs[name])), f"{name} nonzero unsupported"
    wg = w_qkv * g1[None, :]  # fold LN gamma into qkv weights
    wg3 = wg.reshape(NH, 3 * CH, C)
    wq = wg3[:, 0:CH, :]
    wk = wg3[:, CH:2 * CH, :]
    wv_ = wg3[:, 2 * CH:3 * CH, :]
    # qk columns interleaved per head: j = h*96 + (0..47 q | 48..95 k)
    wqk = np.concatenate([wq, wk], axis=1).reshape(2 * C, C)
    if FP8_QK:
        wqk_q = (wqk * WSC).astype(f8)
        wqk_t = np.zeros((4 * P, 2 * C), f8)
        wqk_t[0:C] = np.ascontiguousarray(wqk_q.T)
        # u from the actually-quantized weights so the mean fold matches
        u_qk = (wqk_q.astype(np.float32) / WSC).sum(axis=1)[None, :]
    else:
        wqk_t = np.ascontiguousarray(wqk.T).astype(bf)  # [384, 768]
        u_qk = wqk.sum(axis=1)[None, :]  # [1, 768]
    if FP8_QK:
        u_qk = u_qk * (XSC * WSC)
    usb = np.broadcast_to(u_qk, (P, 2 * C))
    wv_t = np.ascontiguousarray(wv_.transpose(1, 0, 2))  # [48, NH, 384]
    wpj48 = np.ascontiguousarray(
        np.asarray(inputs["w_proj"], np.float32).T.reshape(NH, CH, C)
        .transpose(1, 0, 2))
    w1g = np.asarray(inputs["w_ffn1"], np.float32) * g2[None, :]
    w1_t = np.zeros((4 * P, HID), np.float32)
    w1_t[0:C] = w1g.T
    w2_t = np.ascontiguousarray(np.asarray(inputs["w_ffn2"], np.float32).T)
    ls = np.asarray(inputs["logit_scale"], np.float32).reshape(NH)
    scale_row = np.exp(np.minimum(ls, LOGIT_MAX))[None, :]
    return dict(
        wqk_t=wqk_t,
        usb=np.ascontiguousarray(usb).astype(bf),
        wv=wv_t.astype(bf),
        wpj48=(wpj48 * WSC).astype(f8) if FP8_G else wpj48.astype(bf),
        w1_t=(w1_t * 64.0).astype(f8), w2_t=(w2_t * 64.0).astype(f8),
        scale_row=np.ascontiguousarray(scale_row).astype(np.float32))


def _make_in_maps(inputs):
    import ml_dtypes
    x = np.asarray(inputs["x"], np.float32).reshape(B, C, N)
    xb = x.astype(ml_dtypes.bfloat16)
    wmap = _prep_weights(inputs)
    in_maps = []
    for c in range(NCORES):
        m = dict(wmap)
        m["xs"] = np.ascontiguousarray(xb[c * BPC:(c + 1) * BPC])
        if FP8_QK:
            m["xs8"] = np.ascontiguousarray(
                (x[c * BPC:(c + 1) * BPC] * XSC).astype(
                    ml_dtypes.float8_e4m3))
        in_maps.append(m)
    return in_maps


def kernel(**inputs):
    from concourse.bass_utils import run_bass_kernel_spmd

    if "nc" not in _CACHE:
        _CACHE["nc"] = _build_nc()
    nc = _CACHE["nc"]
    in_maps = _make_in_maps(inputs)
    res = run_bass_kernel_spmd(nc, in_maps, list(range(NCORES)))
    out = np.concatenate(
        [np.asarray(r["out"], np.float32) for r in res.results], axis=0)
    return out.reshape(B, C, 64, 64)


# revision 32
# speedup vs baseline: 1.1684x; 1.1684x over previous
"""Trainium2 Bass kernel for nn_CATransformer1 (XCiT-style channel-attention block).

Sharding: data-parallel over batch. 16 images / 8 cores = 2 images per core.
Weights replicated; no collectives.

V3 design (matmul cost on TRN2 = out-free-size x pe_cycle x cycles_per_row,
independent of K/M; fp8e4+DoubleRow = 0.5 cycles/row):
  - Gram trick: per head, one matmul lhsT=rhs=[q|k] accumulates S=q^T k AND
    both norm^2 diagonals; kills qksq (DVE), k-norm row (F=384/chunk) and
    q-norm (F=1 x8/chunk) entirely.  Diagonals extracted once per image via
    tensor_tensor_reduce against identity.
  - LN1 mean fold applied at eviction on DVE (scalar_tensor_tensor with a
    host-broadcast usb=ones(x)u) instead of K=1 rank-1 matmuls on the PE.
  - All ones(x)row PE broadcasts replaced with gpsimd partition_broadcast.
  - FFN1 zero-padded to 4 K-subtiles -> pure fp8 DoubleRow.
  - Optional fp8 qkT and fp8 G-apply (DoubleRow, zero-padded 4th subtile).
  - reciprocal_approx_fast everywhere (LN rstd, norms, softmax sum).
  - Softmax max-subtraction dropped (|logits| <= ~10, exp safe in f32).
  - y computed in-place into the SBUF-resident x (no HBM re-read in B1);
    FFN (B2) chunks interleaved into the B1 pipeline to keep the PE fed.
"""

import numpy as np

B, C, NH, CH, N, HID = 16, 384, 8, 48, 4096, 1536
NCORES = 8
BPC = B // NCORES  # images per core
P = 128
KS = C // P   # 3 k-subtiles for C
KH = HID // P  # 12 k-subtiles for HID
NT = N // P   # 32 pixel chunks (phase A)
FG = 512      # phase B pixel chunk
NFG = N // FG
LOGIT_MAX = float(np.log(1.0 / 0.01))
EPS_LN = 1e-5
EPS_NORM = 1e-12

FP8_QK = False   # qkT matmul in fp8 DoubleRow (x_f8 = 8x, wqk_f8 = 64w)
FP8_G = False    # G-apply + s3 in fp8 DoubleRow

XSC = 8.0        # x fp8 scale
WSC = 64.0       # weight fp8 scale
GSC = 256.0      # G fp8 scale

_CACHE = {}


def _patch_tile_drain():
    """Walrus in this env rejects >1 sync-wait on the kernel-tail Drain
    (CTRL_NO_STRUCT setupSyncWait).  Split the waits across a chain of
    drain instructions, one wait each.  Idempotent, in-process only."""
    import concourse.tile as tile
    from concourse import mybir
    from concourse.vector_clock import ScopedClock

    if getattr(tile.TileContext._drain_and_barrier, "_split_patch", False):
        return

    def _split_drain(self, tick_clock, wait_clock):
        drain_inst = self.nc.sync.drain()
        wait_clock.add_sem_waits(
            drain_inst.ins, ScopedClock({None: tick_clock.global_clock}))
        si = drain_inst.ins.sync_info
        if si is not None and si.on_wait and len(si.on_wait) > 1:
            waits = list(si.on_wait)
            si.on_wait = waits[:1]
            for w in waits[1:]:
                d2 = self.nc.sync.drain()
                d2.ins.sync_info = mybir.SyncInfo(on_wait=[w], on_update=[])
        self.nc.all_engine_barrier()
        popped = self.nc._tile_sem_poison_stack.pop()
        assert popped is self._sem_poison
        self.nc.clear_and_free_semaphores(list(self.sems.allocated().values()))
        self.nc.all_engine_barrier()

    _split_drain._split_patch = True
    tile.TileContext._drain_and_barrier = _split_drain


def _split_waits(nc, max_waits=1):
    """This walrus build rejects instructions carrying more than one sync
    wait ('Too many sync wait commands' / 'ISA wrong length').  Move extra
    waits onto same-engine NoOps inserted immediately before."""
    from concourse import mybir

    n = 0
    for fn in nc.m.functions:
        for blk in fn.blocks:
            out = []
            for inst in blk.instructions:
                si = inst.sync_info
                # custom-DVE InstISA can't carry any sync commands at all
                mw = 0 if isinstance(inst, mybir.InstISA) else max_waits
                if si is not None and si.on_wait and len(si.on_wait) > mw:
                    waits = list(si.on_wait)
                    keep = waits[-mw:] if mw else []
                    for w in waits[:len(waits) - mw]:
                        n += 1
                        nop = mybir.InstNoOp(
                            name=f"I-wsplit-{n}", ins=[], outs=[])
                        nop.engine = inst.engine
                        nop.sync_info = mybir.SyncInfo(
                            on_wait=[w], on_update=[])
                        out.append(nop)
                    si.on_wait = keep
                out.append(inst)
                if (isinstance(inst, mybir.InstISA) and si is not None
                        and si.on_update):
                    n += 1
                    nop = mybir.InstNoOp(name=f"I-usplit-{n}", ins=[], outs=[])
                    nop.engine = inst.engine
                    nop.sync_info = mybir.SyncInfo(
                        on_wait=[], on_update=list(si.on_update))
                    out.append(nop)
                    si.on_update = []
            blk.instructions = out
    return nc


def _build_nc():
    import concourse.bass as bass
    import concourse.tile as tile
    from concourse import mybir

    dt = mybir.dt
    AF = mybir.ActivationFunctionType
    ALU = mybir.AluOpType
    AX = mybir.AxisListType
    from concourse.masks import make_identity

    f32 = dt.float32
    bf16 = dt.bfloat16
    f8 = dt.float8e4

    _patch_tile_drain()
    nc = bass.Bass()

    xs = nc.declare_dram_parameter("xs", [BPC, C, N], bf16, isOutput=False)
    if FP8_QK:
        xs8 = nc.declare_dram_parameter("xs8", [BPC, C, N], f8, isOutput=False)
        wqk_t = nc.declare_dram_parameter("wqk_t", [4 * P, 2 * C], f8,
                                          isOutput=False)
    else:
        wqk_t = nc.declare_dram_parameter("wqk_t", [C, 2 * C], bf16,
                                          isOutput=False)
    usb_d = nc.declare_dram_parameter("usb", [P, 2 * C], bf16, isOutput=False)
    wv = nc.declare_dram_parameter("wv", [CH, NH, C], bf16, isOutput=False)
    if FP8_G:
        wpj48 = nc.declare_dram_parameter("wpj48", [CH, NH, C], f8,
                                          isOutput=False)
    else:
        wpj48 = nc.declare_dram_parameter("wpj48", [CH, NH, C], bf16,
                                          isOutput=False)
    w1_t = nc.declare_dram_parameter("w1_t", [4 * P, HID], f8, isOutput=False)
    w2_t = nc.declare_dram_parameter("w2_t", [HID, C], f8, isOutput=False)
    scale_row = nc.declare_dram_parameter("scale_row", [1, NH], f32,
                                          isOutput=False)
    out_d = nc.declare_dram_parameter("out", [BPC, C, N], bf16, isOutput=True)
    # DRAM scratch for partition-broadcast round trips: a row written here
    # comes back replicated across 128 partitions via a 0-stride DMA read
    rows_d = nc.dram_tensor("rows_sc", [BPC, 2, N], bf16)   # (rrow, mrnrow)
    rows2_d = nc.dram_tensor("rows2_sc", [2, N], bf16)      # LN2 (m2, r2)

    qk_dt = f8 if FP8_QK else bf16
    g_dt = f8 if FP8_G else bf16
    KSQ = 4 if FP8_QK else KS   # qkT lhs subtiles incl zero pad
    KSG = 4 if FP8_G else KS

    with tile.TileContext(nc) as tc:
        with (
            tc.tile_pool(name="consts", bufs=1) as consts,
            tc.tile_pool(name="ximg", bufs=2) as xpool,
            tc.tile_pool(name="qkp", bufs=2) as qkpool,
            tc.tile_pool(name="attn", bufs=2) as apool,
            tc.tile_pool(name="scr", bufs=3) as scr,
            tc.tile_pool(name="bwork", bufs=2) as bw,
            tc.tile_pool(name="pb", bufs=6, space="PSUM") as ps,
            tc.tile_pool(name="acc", bufs=1, space="PSUM") as psacc,
        ):
            def bcast_read(dst, dram_row, parts):
                src = bass.AP(
                    tensor=dram_row.tensor, offset=dram_row.offset,
                    ap=[[0, parts]] + [list(d) for d in dram_row.ap[-1:]])
                nc.gpsimd.dma_start(dst, src)

            # ---------------- constants ----------------
            ones_col = consts.tile([P, 1], bf16, tag="onescol")
            nc.vector.memset(ones_col[:], 1.0)
            ones_row = consts.tile([1, P], bf16, tag="onesrow")
            nc.vector.memset(ones_row[:], 1.0)
            identb = consts.tile([P, P], bf16, tag="identb")
            make_identity(nc, identb[:])
            schb = consts.tile([CH, NH], f32, tag="schb")
            bcast_read(schb[:], scale_row[0, :], parts=CH)

            xs_r = xs.rearrange("b (s p) n -> b p s n", p=P)
            out_r = out_d.rearrange("b (s p) n -> b p s n", p=P)

            # x images: first 512-px chunk of img0 goes first (gates phase A)
            x_tiles, rowpairs = [], []
            for img in range(BPC):
                x_sb = xpool.tile([P, KS, N], bf16, tag="x")
                x_tiles.append(x_sb)
                # rows[0] = rstd, rows[1] = -mean*rstd (per pixel)
                rows = xpool.tile([1, 2, N], bf16, tag="rows")
                rowpairs.append(rows)
            nc.sync.dma_start(x_tiles[0][:, :, 0:512], xs_r[0][:, :, 0:512])

            # qkv weights split across queues for fast availability
            wqk_sb = consts.tile([P, KSQ, 2 * C], qk_dt, tag="wqk")
            wqk_r = wqk_t.rearrange("(s p) f -> p s f", p=P)
            nc.scalar.dma_start(wqk_sb[:, 0, :], wqk_r[:, 0, :])
            nc.gpsimd.dma_start(wqk_sb[:, 1, :], wqk_r[:, 1, :])
            nc.gpsimd.dma_start(wqk_sb[:, 2, :], wqk_r[:, 2, :])
            if FP8_QK:
                nc.scalar.dma_start(wqk_sb[:, 3, :], wqk_r[:, 3, :])
            usb = consts.tile([P, 2 * C], bf16, tag="usb")
            nc.scalar.dma_start(usb[:], usb_d[:])

            # fp8 copies of x (4th subtile zeroed once; pairs with zero rows
            # in wqk so garbage would be harmless, but keep it clean)
            if FP8_QK:
                xs8_r = xs8.rearrange("b (s p) n -> b p s n", p=P)
                x8_tiles = []
                for img in range(BPC):
                    x8 = xpool.tile([P, 4, N], f8, tag="x8")
                    nc.gpsimd.memset(x8[:, 3, :], 0.0)
                    for i in range(4):
                        sl = slice(i * 1024, (i + 1) * 1024)
                        nc.scalar.dma_start(x8[:, 0:3, sl], xs8_r[img][:, :, sl])
                    x8_tiles.append(x8)

            # rest of x img0 + img1
            for img in range(BPC):
                for i in range(8):
                    if img == 0 and i == 0:
                        continue
                    sl = slice(i * 512, (i + 1) * 512)
                    nc.sync.dma_start(x_tiles[img][:, :, sl],
                                      xs_r[img][:, :, sl])

            wv_sb = consts.tile([CH, NH, C], bf16, tag="wv")
            nc.gpsimd.dma_start(wv_sb[:], wv[:])
            wpj_sb = consts.tile([CH, NH, C], g_dt, tag="wpj")
            nc.gpsimd.dma_start(wpj_sb[:], wpj48[:])
            w1_sb = consts.tile([P, 4, HID], f8, tag="w1")
            nc.gpsimd.dma_start(w1_sb[:], w1_t.rearrange("(s p) f -> p s f", p=P))
            w2_sb = consts.tile([P, KH, C], f8, tag="w2")
            nc.gpsimd.dma_start(w2_sb[:], w2_t.rearrange("(s p) f -> p s f", p=P))

            # yn ring: LN2 output only lives from ln2_apply(f) to b2(f), so a
            # 3-deep ring of single-chunk tiles suffices.  4th subtile is the
            # DoubleRow zero pad, written once (never touched again).
            yns = []
            for i in range(4):
                ynt = bw.tile([P, 4, FG], f8, tag="yn", bufs=4,
                              name=f"ynring{i}")
                nc.vector.memset(ynt[:, 3, :], 0.0)
                yns.append(ynt)

            # ---------------- phase A ----------------
            def alloc_acc():
                # per-head Gram accumulator [q|k]^T[q|k], 128-col stride so
                # every head's 96x96 block stays inside one PSUM bank
                return psacc.tile([2 * CH, NH, P], f32, tag="accg",
                                  name="accg")

            def phase_a(img, accg, interleave=()):
                x_sb = x_tiles[img]
                rows = rowpairs[img]

                def stats_mm2(t):
                    """Batched LN1 stats for chunks t and t+1."""
                    sl = slice(t * P, (t + 2) * P)
                    xsq = scr.tile([P, KS, 2 * P], bf16, tag="xsq", bufs=2)
                    nc.gpsimd.tensor_mul(xsq[:], x_sb[:, :, sl], x_sb[:, :, sl])
                    pstat = ps.tile([P, 2, 2], f32, tag="pb")
                    for cp in range(2):
                        csl = slice((t + cp) * P, (t + cp + 1) * P)
                        for s in range(KS):
                            nc.tensor.matmul(
                                pstat[:, cp, 0:1], x_sb[:, s, csl], ones_col[:],
                                start=(s == 0), stop=(s == KS - 1))
                        for s in range(KS):
                            nc.tensor.matmul(
                                pstat[:, cp, 1:2],
                                xsq[:, s, cp * P:(cp + 1) * P], ones_col[:],
                                start=(s == 0), stop=(s == KS - 1))
                    # stat2 cols: 0 = rstd, 32 = -m*rstd (transposed to rows)
                    stat2 = scr.tile([P, 2, 33], bf16, tag="stat2", bufs=2)
                    mcol = scr.tile([P, 2], f32, tag="mcol", bufs=2)
                    vcol = scr.tile([P, 2], f32, tag="vcol")
                    msq = scr.tile([P, 2], f32, tag="msq")
                    rcol = scr.tile([P, 2], f32, tag="rcol", bufs=2)
                    mrn = scr.tile([P, 2], f32, tag="mrn", bufs=2)
                    nc.scalar.activation(mcol[:], pstat[:, :, 0], AF.Copy,
                                         scale=-1.0 / C)
                    nc.vector.tensor_scalar(
                        vcol[:], pstat[:, :, 1], 1.0 / C, EPS_LN,
                        op0=ALU.mult, op1=ALU.add)
                    nc.scalar.activation(msq[:], mcol[:], AF.Square)
                    nc.vector.tensor_sub(vcol[:], vcol[:], msq[:])
                    nc.scalar.activation(vcol[:], vcol[:], AF.Sqrt)
                    nc.vector.reciprocal_approx_fast(rcol[:], vcol[:])
                    nc.vector.tensor_copy(stat2[:, :, 0], rcol[:])
                    nc.vector.tensor_mul(mrn[:], mcol[:], rcol[:])
                    nc.vector.tensor_copy(stat2[:, :, 32], mrn[:])
                    if FP8_QK:
                        # usb is host-scaled by XSC*WSC to match pa's scale;
                        # the final r-scale compensates
                        rcolq = scr.tile([P, 2], f32, tag="rcolq", bufs=2)
                        nc.vector.tensor_scalar_mul(
                            rcolq[:], rcol[:], 1.0 / (XSC * WSC))
                        return stat2, rcolq, mcol
                    return stat2, rcol, mcol

                def stats_tr(t, stat2, cp):
                    sl = slice((t + cp) * P, (t + cp + 1) * P)
                    ptr = ps.tile([33, P], bf16, tag="pb")
                    nc.tensor.transpose(ptr[:], stat2[:, cp, :], identb[:])
                    nc.scalar.copy(rows[0:1, 0, sl], ptr[0:1, :])
                    nc.scalar.copy(rows[0:1, 1, sl], ptr[32:33, :])

                pend = None
                cur = stats_mm2(0)
                stats_tr(0, cur[0], 0)
                stats_tr(0, cur[0], 1)
                nxt = None
                for t in range(NT):
                    if 1 <= t <= len(interleave):
                        interleave[t - 1]()
                    sl = slice(t * P, (t + 1) * P)
                    cp = t % 2
                    rcol_t = cur[1][:, cp:cp + 1]
                    mcol_t = cur[2][:, cp:cp + 1]
                    if t % 2 == 0 and t + 2 < NT:
                        nxt = stats_mm2(t + 2)
                    # qkT x-part into PSUM (two banks)
                    pa1 = ps.tile([P, 512], f32, tag="pb")
                    pa2 = ps.tile([P, 256], f32, tag="pb")
                    if FP8_QK:
                        x8 = x8_tiles[img]
                        for sp in range(2):
                            ss = slice(2 * sp, 2 * sp + 2)
                            nc.tensor.matmul(
                                pa1[:], x8[:, ss, sl], wqk_sb[:, ss, 0:512],
                                start=(sp == 0), stop=(sp == 1),
                                perf_mode=mybir.MatmulPerfMode.DoubleRow)
                        for sp in range(2):
                            ss = slice(2 * sp, 2 * sp + 2)
                            nc.tensor.matmul(
                                pa2[:], x8[:, ss, sl], wqk_sb[:, ss, 512:768],
                                start=(sp == 0), stop=(sp == 1),
                                perf_mode=mybir.MatmulPerfMode.DoubleRow)
                    else:
                        for s in range(KS):
                            nc.tensor.matmul(
                                pa1[:], x_sb[:, s, sl], wqk_sb[:, s, 0:512],
                                start=(s == 0), stop=(s == KS - 1))
                        for s in range(KS):
                            nc.tensor.matmul(
                                pa2[:], x_sb[:, s, sl], wqk_sb[:, s, 512:768],
                                start=(s == 0), stop=(s == KS - 1))
                    if t % 2 == 0 and t + 2 < NT:
                        stats_tr(t + 2, nxt[0], 0)
                        stats_tr(t + 2, nxt[0], 1)
                    # deferred S/norm accumulation from previous chunk
                    if pend is not None:
                        _emit_s(accg, pend[0], pend[1])
                    # eviction: qk = (pa + (-m)*u) * r
                    # z = usb*(-m) + pa on DVE; qk = z*r on ACT
                    z = qkpool.tile([P, 2 * C], bf16, tag="z")
                    qk = qkpool.tile([P, 2 * C], bf16, tag="qk")
                    nc.vector.scalar_tensor_tensor(
                        z[:, 0:512], usb[:, 0:512], mcol_t, pa1[:],
                        op0=ALU.mult, op1=ALU.add)
                    nc.vector.scalar_tensor_tensor(
                        z[:, 512:768], usb[:, 512:768], mcol_t, pa2[:],
                        op0=ALU.mult, op1=ALU.add)
                    nc.scalar.activation(qk[:], z[:], AF.Copy, scale=rcol_t)
                    pend = (qk, t)
                    if t % 2 == 1:
                        cur = nxt
                _emit_s(accg, pend[0], pend[1])
                # stage the per-pixel LN1 rows in DRAM for phase-B broadcasts
                nc.scalar.dma_start(rows_d[img], rows[0:1, :, :])

            def _emit_s(accg, qk, t):
                st, sp = (t == 0), (t == NT - 1)
                for h in range(NH):
                    o = h * 2 * CH
                    nc.tensor.matmul(
                        accg[:, h, 0:2 * CH],
                        qk[:, o:o + 2 * CH], qk[:, o:o + 2 * CH],
                        start=st, stop=sp)

            # ---------------- attention stages ----------------
            def attn_stages(img, accg):
                st = {}

                def s0a():  # norms (no PE work: keeps the PE queue clear)
                    # free the PSUM accumulator early: S copy + diag extract
                    sraw = apool.tile([CH, NH, CH], f32, tag="sraw", bufs=1)
                    s_v = accg[0:CH, :, CH:2 * CH]
                    nc.vector.tensor_copy(sraw[:], s_v)
                    nsq = apool.tile([2 * CH, NH], f32, tag="nsq", bufs=1)
                    dscr = apool.tile([2 * CH, 2 * CH], bf16, tag="dscr",
                                      bufs=1)
                    for h in range(NH):
                        nc.vector.tensor_mul(
                            dscr[:], accg[:, h, 0:2 * CH],
                            identb[0:2 * CH, 0:2 * CH])
                        nc.vector.reduce_sum(nsq[:, h:h + 1], dscr[:],
                                             axis=AX.X)
                    rinv = apool.tile([2 * CH, NH], f32, tag="rinv", bufs=1)
                    nc.scalar.activation(rinv[:], nsq[:], AF.Sqrt)
                    nc.vector.tensor_scalar_max(rinv[:], rinv[:], EPS_NORM)
                    nc.vector.reciprocal_approx_fast(rinv[:], rinv[:])
                    rq = apool.tile([CH, NH], f32, tag="rq", bufs=1)
                    nc.vector.tensor_mul(rq[:], rinv[0:CH, :], schb[:])
                    # engines can't address a partition window based at 48;
                    # DMA the k-half of rinv down to base 0
                    rkb32 = apool.tile([CH, NH], f32, tag="rkb32", bufs=1)
                    nc.gpsimd.dma_start(rkb32[:], rinv[CH:2 * CH, :])
                    rkb = apool.tile([CH, NH], bf16, tag="rkb", bufs=1)
                    nc.vector.tensor_copy(rkb[:], rkb32[:])
                    st["rq"], st["rkb"], st["sraw"] = rq, rkb, sraw

                def s0b():  # rk row flatten + scaled S + softmax -> sSb
                    rq, rkb, sraw = st["rq"], st["rkb"], st["sraw"]
                    # flatten rk column [48, NH] to one row [1, 384] via 8
                    # single-column transposes, then a K=1 bcast matmul
                    prow = ps.tile([1, NH, CH], bf16, tag="pb")
                    for h in range(NH):
                        nc.tensor.transpose(
                            prow[:, h, :], rkb[:, h:h + 1], identb[0:CH, 0:CH])
                    rkrow = apool.tile([1, NH * CH], bf16, tag="rkrow", bufs=1)
                    nc.scalar.copy(rkrow[:], prow.rearrange("p h c -> p (h c)"))
                    rkps = ps.tile([CH, NH, CH], f32, tag="pb")
                    nc.tensor.matmul(
                        rkps.rearrange("p h c -> p (h c)"),
                        ones_row[0:1, 0:CH], rkrow[0:1, :],
                        start=True, stop=True)
                    sS = apool.tile([CH, NH, CH], f32, tag="sS", bufs=1)
                    nc.vector.tensor_mul(
                        sS[:], sraw[:], rq[:, :, None].to_broadcast(
                            (CH, NH, CH)))
                    nc.vector.tensor_mul(sS[:], sS[:], rkps[:])
                    # |logits| <= ~10: exp directly in f32, no max-sub
                    nc.scalar.activation(sS[:], sS[:], AF.Exp)
                    esum = apool.tile([CH, NH], f32, tag="esum", bufs=1)
                    nc.vector.reduce_sum(esum[:], sS[:], axis=AX.X)
                    nc.vector.reciprocal_approx_fast(esum[:], esum[:])
                    sSb = apool.tile([CH, NH, CH], bf16, tag="sSb", bufs=1)
                    nc.vector.tensor_mul(
                        sSb[:], sS[:],
                        esum[:, :, None].to_broadcast((CH, NH, CH)))
                    st["sSb"] = sSb

                def s1():  # transpose attn per head
                    pt8 = ps.tile([CH, NH, CH], bf16, tag="pb")
                    for h in range(NH):
                        nc.tensor.transpose(
                            pt8[:, h, :], st["sSb"][:, h, :], identb[0:CH, 0:CH])
                    atT = apool.tile([CH, NH, CH], bf16, tag="atT", bufs=1)
                    nc.vector.tensor_copy(atT[:], pt8[:])
                    st["atT"] = atT

                def s2():  # awv_h = attn_h @ Wv_h
                    awv = apool.tile([CH, NH, C], g_dt, tag="awv", bufs=1)
                    for h in range(NH):
                        paw = ps.tile([CH, C], f32, tag="pb")
                        nc.tensor.matmul(
                            paw[:], st["atT"][:, h, :], wv_sb[:, h, :],
                            start=True, stop=True)
                        sc = WSC if FP8_G else 1.0
                        if h % 2 == 0:
                            nc.vector.tensor_scalar_mul(awv[:, h, :], paw[:], sc)
                        else:
                            nc.scalar.activation(awv[:, h, :], paw[:], AF.Copy,
                                                 scale=sc)
                    st["awv"] = awv

                def s3():  # G^T  (fp8: awv*64 x wpj*64 -> evict * 256/4096)
                    gt_sb = apool.tile([P, KSG, C], g_dt, tag="gt")
                    if FP8_G:
                        nc.gpsimd.memset(gt_sb[:, 3, :], 0.0)
                    for j in range(KS):
                        pgt = ps.tile([P, C], f32, tag="pb")
                        if FP8_G:
                            for hp in range(NH // 2):
                                hs = slice(2 * hp, 2 * hp + 2)
                                nc.tensor.matmul(
                                    pgt[:], st["awv"][:, hs, j * P:(j + 1) * P],
                                    wpj_sb[:, hs, :],
                                    start=(hp == 0), stop=(hp == NH // 2 - 1),
                                    perf_mode=mybir.MatmulPerfMode.DoubleRow)
                        else:
                            for h in range(NH):
                                nc.tensor.matmul(
                                    pgt[:], st["awv"][:, h, j * P:(j + 1) * P],
                                    wpj_sb[:, h, :], start=(h == 0),
                                    stop=(h == NH - 1))
                        sc = GSC / (WSC * WSC) if FP8_G else 1.0
                        if j % 2 == 0:
                            nc.vector.tensor_scalar_mul(gt_sb[:, j, :], pgt[:],
                                                        sc)
                        else:
                            nc.scalar.activation(gt_sb[:, j, :], pgt[:],
                                                 AF.Copy, scale=sc)
                    st["gt"] = gt_sb

                def s4():  # uG column [128, KS] f32 (per out-channel block)
                    ugps = ps.tile([P, KS], f32, tag="pb")
                    onesb = ones_col
                    for j in range(KS):
                        for s in range(KS):
                            nc.tensor.matmul(
                                ugps[:, j:j + 1],
                                st["gt"][:, s, j * P:(j + 1) * P], onesb[:],
                                start=(s == 0), stop=(s == KS - 1))
                    ug = apool.tile([P, KS], f32, tag="ug")
                    # ug holds the true (unscaled) uG column
                    sc = 1.0 / GSC if FP8_G else 1.0
                    nc.vector.tensor_scalar_mul(ug[:], ugps[:], sc)
                    st["ug"] = ug

                return [s0a, s0b, s1, s2, s3, s4], st

            # ---------------- phase B ----------------
            f8sc = 64.0  # host scales w1/w2 by 64
            g_inv = 1.0 / (GSC * XSC) if FP8_G else 1.0

            def bcast_sb(dst, row_ap, engine=None):
                """DRAM row -> SBUF replicated across partitions."""
                src = bass.AP(
                    tensor=row_ap.tensor, offset=row_ap.offset,
                    ap=[[0, dst.shape[0]]] + [list(d) for d in row_ap.ap])
                (engine or nc.sync).dma_start(dst, src)

            def b1_bcast(img, f):
                """Prefetch the (rstd, -m*rstd) broadcast for chunk f."""
                sl = slice(f * FG, (f + 1) * FG)
                brow = bw.tile([P, 2, FG], bf16, tag="brow", bufs=3)
                bcast_sb(brow, rows_d[img, :, sl], engine=nc.scalar)
                return brow

            def b1_chunk(img, st, f, brow):
                """Attn-branch apply + residual (in place into x) + ysq."""
                x_sb = x_tiles[img]
                gt, ug = st["gt"], st["ug"]
                sl = slice(f * FG, (f + 1) * FG)
                rb = brow[:, 0, :]    # rstd
                mrb = brow[:, 1, :]   # -m*rstd
                ab = bw.tile([P, KS, FG], bf16, tag="ab", bufs=1)
                for j in range(KS):
                    pg = ps.tile([P, FG], f32, tag="pb", name=f"pg{j}")
                    if FP8_G:
                        x8 = x8_tiles[img]
                        for sp in range(2):
                            ss = slice(2 * sp, 2 * sp + 2)
                            nc.tensor.matmul(
                                pg[:], gt[:, ss, j * P:(j + 1) * P],
                                x8[:, ss, sl], start=(sp == 0), stop=(sp == 1),
                                perf_mode=mybir.MatmulPerfMode.DoubleRow)
                    else:
                        for s in range(KS):
                            nc.tensor.matmul(
                                pg[:], gt[:, s, j * P:(j + 1) * P],
                                x_sb[:, s, sl], start=(s == 0),
                                stop=(s == KS - 1))
                    # pgr = (pg*g_inv)*rstd ; ab = mrb*uG[j] + pgr
                    pgr = bw.tile([P, FG], bf16, tag="pgr", bufs=3)
                    nc.vector.scalar_tensor_tensor(
                        pgr[:], pg[:], g_inv, rb,
                        op0=ALU.mult, op1=ALU.mult)
                    nc.vector.scalar_tensor_tensor(
                        ab[:, j, :], mrb, ug[:, j:j + 1], pgr[:],
                        op0=ALU.mult, op1=ALU.add)
                # y = x + ab, in place
                nc.gpsimd.tensor_add(x_sb[:, :, sl], x_sb[:, :, sl], ab[:])
                ysq = bw.tile([P, KS, FG], bf16, tag="ysq", bufs=2)
                nc.gpsimd.tensor_mul(ysq[:], x_sb[:, :, sl], x_sb[:, :, sl])
                return ysq

            def ln2_stats(img, f, ysq):
                x_sb = x_tiles[img]
                sl = slice(f * FG, (f + 1) * FG)
                p2a = ps.tile([1, FG], f32, tag="pb")
                p2b = ps.tile([1, FG], f32, tag="pb")
                for s in range(KS):
                    nc.tensor.matmul(
                        p2a[:], ones_col[:], x_sb[:, s, sl],
                        start=(s == 0), stop=(s == KS - 1))
                for s in range(KS):
                    nc.tensor.matmul(
                        p2b[:], ones_col[:], ysq[:, s, :],
                        start=(s == 0), stop=(s == KS - 1))
                m2r2 = scr.tile([1, 2, FG], bf16, tag="m2r2", bufs=2)
                nc.scalar.activation(m2r2[:, 0, :], p2a[:], AF.Copy,
                                     scale=-1.0 / C)
                vrow = scr.tile([1, FG], f32, tag="vrow", bufs=2)
                nc.vector.tensor_scalar(
                    vrow[:], p2b[:], 1.0 / C, EPS_LN, op0=ALU.mult, op1=ALU.add)
                msq = scr.tile([1, FG], f32, tag="msqr", bufs=2)
                nc.scalar.activation(msq[:], m2r2[:, 0, :], AF.Square)
                nc.vector.tensor_sub(vrow[:], vrow[:], msq[:])
                nc.scalar.activation(vrow[:], vrow[:], AF.Sqrt)
                r2f = scr.tile([1, FG], f32, tag="r2f", bufs=2)
                nc.vector.reciprocal_approx_fast(r2f[:], vrow[:])
                nc.scalar.copy(m2r2[:, 1, :], r2f[:])
                # round-trip through DRAM for the partition broadcast; write
                # and read are on the same queue so FIFO order is preserved
                nc.scalar.dma_start(rows2_d[:, sl], m2r2[0:1, :, :])
                brow2 = bw.tile([P, 2, FG], bf16, tag="brow2", bufs=3)
                bcast_sb(brow2, rows2_d[:, sl], engine=nc.scalar)
                return brow2

            def ln2_apply(img, f, brow2):
                x_sb = x_tiles[img]
                sl = slice(f * FG, (f + 1) * FG)
                yn = yns[f % 4]
                mbc = brow2[:, 0, :]
                rbc = brow2[:, 1, :]
                t3 = bw.tile([P, KS, FG], bf16, tag="t3", bufs=1)
                for s in range(KS):
                    nc.gpsimd.tensor_add(t3[:, s, :], x_sb[:, s, sl], mbc)
                for s in range(KS):
                    nc.gpsimd.tensor_mul(yn[:, s, :], t3[:, s, :], rbc)

            def b2_chunk(img, f):
                """One FFN chunk in fp8 DoubleRow + residual + store."""
                x_sb = x_tiles[img]
                sl = slice(f * FG, (f + 1) * FG)
                yn = yns[f % 4]
                h_sb = bw.tile([P, KH, FG], f8, tag="h", bufs=2)
                for mh in range(KH):
                    ph = ps.tile([P, FG], f32, tag="pb")
                    for sp in range(2):
                        ss = slice(2 * sp, 2 * sp + 2)
                        nc.tensor.matmul(
                            ph[:], w1_sb[:, ss, mh * P:(mh + 1) * P],
                            yn[:, ss, :], start=(sp == 0), stop=(sp == 1),
                            perf_mode=mybir.MatmulPerfMode.DoubleRow)
                    nc.scalar.activation(
                        h_sb[:, mh, :], ph[:], AF.Gelu, scale=1.0 / f8sc)
                o_sb = bw.tile([P, KS, FG], bf16, tag="o", bufs=2)
                for mo in range(KS):
                    po = ps.tile([P, FG], f32, tag="pb")
                    for sp in range(KH // 2):
                        nc.tensor.matmul(
                            po[:], w2_sb[:, 2 * sp:2 * sp + 2,
                                         mo * P:(mo + 1) * P],
                            h_sb[:, 2 * sp:2 * sp + 2, :],
                            start=(sp == 0), stop=(sp == KH // 2 - 1),
                            perf_mode=mybir.MatmulPerfMode.DoubleRow)
                    nc.vector.scalar_tensor_tensor(
                        o_sb[:, mo, :], po[:], 1.0 / f8sc, x_sb[:, mo, sl],
                        op0=ALU.mult, op1=ALU.add)
                nc.sync.dma_start(out_r[img][:, :, sl], o_sb[:])

            def phase_b_all(sts, pre_brows, interleave=()):
                """Unified B pipeline over both images: stats lag 1,
                apply lag 2, FFN lag 3.  The pipeline slides straight
                through the image boundary so the PE never drains."""
                steps = [(img, f) for img in range(BPC) for f in range(NFG)]
                ns = len(steps)
                ysqs, lrows, brows = [], [], list(pre_brows)
                ivs = list(interleave)
                for s in range(ns + 3):
                    if s + 2 < ns:
                        brows.append(b1_bcast(*steps[s + 2]))
                    if s < ns:
                        img, f = steps[s]
                        ysqs.append(b1_chunk(img, sts[img], f, brows[s]))
                    if s >= 1 and s - 1 < ns:
                        img, f = steps[s - 1]
                        lrows.append(ln2_stats(img, f, ysqs[s - 1]))
                    if s >= 2 and s - 2 < ns:
                        img, f = steps[s - 2]
                        ln2_apply(img, f, lrows[s - 2])
                    if s >= 3 and s - 3 < ns:
                        img, f = steps[s - 3]
                        b2_chunk(img, f)
                    if ivs and s >= 1:
                        ivs.pop(0)()

            # ----------------- schedule -----------------
            acc0 = alloc_acc()
            phase_a(0, acc0)
            stages0, st0 = attn_stages(0, acc0)
            # prefetch img0's first B1 broadcasts while phase A(1) runs
            pre_brows = [b1_bcast(0, 0), b1_bcast(0, 1)]
            accB = alloc_acc()
            phase_a(1, accB, interleave=stages0)
            stages1, st1 = attn_stages(1, accB)
            phase_b_all([st0, st1], pre_brows, interleave=stages1)

    # populate .instr bytes for extended-inst InstISA subclasses
    # (InstPartitionBroadcast / InstTensorTensorReduce / custom-DVE ops);
    # raw Bass skips Bacc.compile()'s codegen_inst_isa_subclasses pass
    from concourse.library_overlay import lower_extended_insts
    lower_extended_insts(nc)
    return _split_waits(nc)


def _prep_weights(inputs):
    import ml_dtypes
    bf = ml_dtypes.bfloat16
    f8 = ml_dtypes.float8_e4m3
    w_qkv = np.asarray(inputs["w_qkv"], np.float32)
    g1 = np.asarray(inputs["g1"], np.float32)
    g2 = np.asarray(inputs["g2"], np.float32)
    for name in ("beta1", "beta2", "b_qkv", "b_proj", "b_ffn1", "b_ffn2"):
        assert not np.any(np.asarray(inputs[name])), f"{name} nonzero unsupported"
    wg = w_qkv * g1[None, :]  # fold LN gamma into qkv weights
    wg3 = wg.reshape(NH, 3 * CH, C)
    wq = wg3[:, 0:CH, :]
    wk = wg3[:, CH:2 * CH, :]
    wv_ = wg3[:, 2 * CH:3 * CH, :]
    # qk columns interleaved per head: j = h*96 + (0..47 q | 48..95 k)
    wqk = np.concatenate([wq, wk], axis=1).reshape(2 * C, C)
    if FP8_QK:
        wqk_q = (wqk * WSC).astype(f8)
        wqk_t = np.zeros((4 * P, 2 * C), f8)
        wqk_t[0:C] = np.ascontiguousarray(wqk_q.T)
        # u from the actually-quantized weights so the mean fold matches
        u_qk = (wqk_q.astype(np.float32) / WSC).sum(axis=1)[None, :]
    else:
        wqk_t = np.ascontiguousarray(wqk.T).astype(bf)  # [384, 768]
        u_qk = wqk.sum(axis=1)[None, :]  # [1, 768]
    if FP8_QK:
        u_qk = u_qk * (XSC * WSC)
    usb = np.broadcast_to(u_qk, (P, 2 * C))
    wv_t = np.ascontiguousarray(wv_.transpose(1, 0, 2))  # [48, NH, 384]
    wpj48 = np.ascontiguousarray(
        np.asarray(inputs["w_proj"], np.float32).T.reshape(NH, CH, C)
        .transpose(1, 0, 2))
    w1g = np.asarray(inputs["w_ffn1"], np.float32) * g2[None, :]
    w1_t = np.zeros((4 * P, HID), np.float32)
    w1_t[0:C] = w1g.T
    w2_t = np.ascontiguousarray(np.asarray(inputs["w_ffn2"], np.float32).T)
    ls = np.asarray(inputs["logit_scale"], np.float32).reshape(NH)
    scale_row = np.exp(np.minimum(ls, LOGIT_MAX))[None, :]
    return dict(
        wqk_t=wqk_t,
        usb=np.ascontiguousarray(usb).astype(bf),
        wv=wv_t.astype(bf),
        wpj48=(wpj48 * WSC).astype(f8) if FP8_G else wpj48.astype(bf),
        w1_t=(w1_t * 64.0).astype(f8), w2_t=(w2_t * 64.0).astype(f8),
        scale_row=np.ascontiguousarray(scale_row).astype(np.float32))


def _make_in_maps(inputs):
    import ml_dtypes
    x = np.asarray(inputs["x"], np.float32).reshape(B, C, N)
    xb = x.astype(ml_dtypes.bfloat16)
    wmap = _prep_weights(inputs)
    in_maps = []
    for c in range(NCORES):
        m = dict(wmap)
        m["xs"] = np.ascontiguousarray(xb[c * BPC:(c + 1) * BPC])
        if FP8_QK:
            m["xs8"] = np.ascontiguousarray(
                (x[c * BPC:(c + 1) * BPC] * XSC).astype(
                    ml_dtypes.float8_e4m3))
        in_maps.append(m)
    return in_maps


def kernel(**inputs):
    from concourse.bass_utils import run_bass_kernel_spmd

    if "nc" not in _CACHE:
        _CACHE["nc"] = _build_nc()
    nc = _CACHE["nc"]
    in_maps = _make_in_maps(inputs)
    res = run_bass_kernel_spmd(nc, in_maps, list(range(NCORES)))
    out = np.concatenate(
        [np.asarray(r["out"], np.float32) for r in res.results], axis=0)
    return out.reshape(B, C, 64, 64)
